# revision 1
# baseline (speedup 1.0000x reference)
"""ConditionGateAttention Trainium2 kernel.

Gated dual-attention block: causal self-attention + cross-attention to a
77-token condition, sigmoid cross-gating, output projection.

  B=2, T=2048, M=77, C=512, H=8 heads, D=64.

Sharding (8 cores): batch x sequence. Core = (b=core//4, j=core%4); each
core owns query chunks {j, 7-j} of 256 rows of its batch (balanced causal
work) and computes K/V for the full batch locally (no collectives).

Layouts: activations kept transposed ([C, tokens]) end-to-end so every
matmul consumes the previous one's output directly (zero on-chip
transposes). Matmul inputs fp16 (full PE rate), fp32 PSUM accumulate.

Masking: the host inspects attn_mask and derives, per query-chunk
position, a program-uniform k-extent (max over cores) plus per-core 0/1
mask tiles multiplied onto the exp output on DVE (one [128,1024] multiply
per masked k-group; SPMD-uniform program, per-core data). padding_mask
becomes a per-partition ACT bias on the cross-attention exp. Softmax
denominators come free from a ones-column appended to V; normalization is
reciprocal + DMA-broadcast + DVE mult. A lag-2 software pipeline keeps PE
busy across the exp latency (QK(g+2) issues before AV(g)).
"""
import numpy as np
import ml_dtypes
from contextlib import ExitStack

import concourse.bass as bass
import concourse.tile as tile
from concourse import bacc, mybir
from concourse import bass_utils

B, T, M, C, H = 2, 2048, 77, 512, 8
D = C // H            # 64
P = 128
KI = C // P           # 4 contraction chunks
PAIRS = H // 2        # 4 head pairs (pair i = heads 2i, 2i+1 = C rows 128i..128i+128)
QC = 256              # query chunk size (2 chunks per core)
NCHUNK = T // QC      # 8 chunks per batch
TQ = 2 * QC           # local queries per core
KT = 128              # k-tile size (partition dim of transposed logits)
GROUP = 4             # k-tiles per logits psum group ([128, 1024] fp32 = 2 banks)
NEG = -30000.0        # mask bias (fp16-representable; exp(-30000+s) == 0 in fp32)
MP = 128              # condition length M=77 zero-padded to 128 on host
DA = D + 1            # V augmented with a ones-column (denominator row)

f16 = mybir.dt.float16
f32 = mybir.dt.float32
AF = mybir.ActivationFunctionType
ALU = mybir.AluOpType

_cache = {}


def _chunks_of_core(j):
    return (j, NCHUNK - 1 - j)


def build_program(ext, bias_slots, has_b, stage=4, repeat=1):
    """ext: per-position k-extent in KT tiles (uniform across cores), rounded
    up to GROUP. bias_slots: list of (pos, slot) needing a bias tile (uniform;
    data per-core). has_b: dict of which projection biases are nonzero.
    stage: 0=io-baseline, 1=projections, 15=kc, 2=+self-attn, 31/32/3=+cross
    pieces, 4=full. repeat: run the compute body N times (timing aid)."""
    key = (tuple(ext), tuple(bias_slots), tuple(sorted(has_b.items())),
           stage, repeat)
    if key in _cache:
        return _cache[key]

    nb = len(bias_slots)
    bias_idx = {ps: n for n, ps in enumerate(bias_slots)}

    nc = bacc.Bacc("TRN2", num_devices=8, debug=False)

    xT_d = nc.dram_tensor("xT", [C, T], f16, kind="ExternalInput").ap()
    xqT_d = nc.dram_tensor("xqT", [C, TQ], f16, kind="ExternalInput").ap()
    cT_d = nc.dram_tensor("cT", [C, MP], f16, kind="ExternalInput").ap()
    w_d = {n: nc.dram_tensor(f"w{n}", [C, C], f16, kind="ExternalInput").ap()
           for n in ["q", "k", "v", "kc", "vc", "g1", "g2", "p"]}
    ident_d = nc.dram_tensor("ident", [P, P], f16, kind="ExternalInput").ap()
    pad_d = nc.dram_tensor("padb", [P, 1], f32, kind="ExternalInput").ap()
    if nb:
        bias_d = nc.dram_tensor("biasm", [nb, P, GROUP * QC], f16, kind="ExternalInput").ap()
    bv_d = {}
    for n in ["q", "k", "kc", "g1", "g2"]:
        if has_b[n]:
            bv_d[n] = nc.dram_tensor(f"b{n}", [P, KI], f32, kind="ExternalInput").ap()
    for n in ["v", "vc", "p"]:
        if has_b[n]:
            bv_d[n] = nc.dram_tensor(f"b{n}", [1, C], f16, kind="ExternalInput").ap()
    out_d = nc.dram_tensor("out", [TQ, C], f32, kind="ExternalOutput").ap()

    def emit(tc, ctx):
        consts = ctx.enter_context(tc.tile_pool(name="consts", bufs=1))
        acts = ctx.enter_context(tc.tile_pool(name="acts", bufs=1))
        work = ctx.enter_context(tc.tile_pool(name="work", bufs=4))
        nrm = ctx.enter_context(tc.tile_pool(name="nrm", bufs=4))
        ps_a = ctx.enter_context(tc.tile_pool(name="ps_a", bufs=2, space="PSUM"))
        ps_b = ctx.enter_context(tc.tile_pool(name="ps_b", bufs=2, space="PSUM"))
        ps_y = ctx.enter_context(tc.tile_pool(name="ps_y", bufs=2, space="PSUM"))

        # ---- load constants/inputs ----
        def chunked(ap):  # [C, n] dram -> [128, 4, n] view
            return ap.rearrange("(o p) n -> p o n", p=P)

        # DMA order matters: the q-projection inputs land first so PE can
        # start while the bulk (xT, V/gate weights, bias tiles) streams in.
        w_sb = {n: consts.tile([P, KI, C], f16, name=f"w{n}") for n in w_d}
        xqT_sb = consts.tile([P, KI, TQ], f16, name="xqT")
        nc.sync.dma_start(xqT_sb[:], chunked(xqT_d))
        nc.sync.dma_start(w_sb["q"][:], chunked(w_d["q"]))
        xT_sb = consts.tile([P, KI, T], f16, name="xT")
        nc.sync.dma_start(xT_sb[:], chunked(xT_d))
        nc.sync.dma_start(w_sb["k"][:], chunked(w_d["k"]))
        cT_sb = consts.tile([P, KI, MP], f16, name="cT")
        nc.sync.dma_start(cT_sb[:], chunked(cT_d))
        for n in ["kc", "v", "vc", "g1", "g2", "p"]:
            nc.sync.dma_start(w_sb[n][:], chunked(w_d[n]))
        ident = consts.tile([P, P], f16, name="ident")
        nc.sync.dma_start(ident[:], ident_d)
        pad_sb = consts.tile([P, 1], f32, name="padb")
        nc.sync.dma_start(pad_sb[:], pad_d)
        if nb:
            bias_sb = consts.tile([P, nb, GROUP * QC], f16, name="biasm")
            nc.sync.dma_start(bias_sb[:], bias_d.rearrange("n p q -> p n q"))
        bv_sb = {}
        for n, d in bv_d.items():
            if n in ("v", "vc", "p"):
                bv_sb[n] = consts.tile([P, C], f16, name=f"b{n}")
                nc.sync.dma_start(bv_sb[n][:],
                                  d[0:1, :].unsqueeze(1).to_broadcast((1, P, C)))
            else:
                bv_sb[n] = consts.tile([P, KI], f32, name=f"b{n}")
                nc.sync.dma_start(bv_sb[n][:], d)

        def dump(srcs):
            for m, src in enumerate(srcs):
                osb = work.tile([P, C], f32, tag="osb")
                w = src.shape[-1]
                if w < C:
                    nc.vector.memset(osb[:], 0.0)
                nc.vector.tensor_copy(osb[:, 0:w], src)
                nc.sync.dma_start(out_d[P * m:P * m + P, :], osb[:])

        if stage == 0:
            # IO-only baseline: same inputs/outputs, no compute
            for m in range(PAIRS):
                osb = work.tile([P, C], f32, tag="osb")
                nc.vector.memset(osb[:], 0.0)
                nc.sync.dma_start(out_d[P * m:P * m + P, :], osb[:])
            return

        # ---- persistent activation tiles ----
        qT_sb = [acts.tile([P, TQ], f16, name=f"qT{i}") for i in range(PAIRS)]
        kT_sb = [acts.tile([P, T], f16, name=f"kT{i}") for i in range(PAIRS)]
        kcT_sb = [acts.tile([P, MP], f16, name=f"kcT{i}") for i in range(PAIRS)]
        v_sb = [acts.tile([P, H * DA], f16, name=f"v{m}") for m in range(T // P)]
        vc_sb = [acts.tile([P, H * DA], f16, name="vc")]
        yT_sb = [acts.tile([P, TQ], f16, name=f"yT{i}") for i in range(PAIRS)]
        ycT_sb = [acts.tile([P, TQ], f16, name=f"ycT{i}") for i in range(PAIRS)]
        g1_sb = [acts.tile([P, TQ], f16, name=f"g1_{o}") for o in range(PAIRS)]
        g2_sb = [acts.tile([P, TQ], f16, name=f"g2_{o}") for o in range(PAIRS)]
        z_sb = [acts.tile([P, TQ], f16, name=f"z{o}") for o in range(PAIRS)]

        # ---- projections ----
        def proj_T(wname, rhs_sb, n_free, out_tiles, free_tile):
            # out[Cout, n] = W.T @ actT ; out_tiles[i] [128, n_free] f16
            for i in range(PAIRS):
                for tt in range(0, n_free, free_tile):
                    fw = min(free_tile, n_free - tt)
                    ps = ps_b.tile([P, 512], f32, tag="psb")
                    for ki in range(KI):
                        nc.tensor.matmul(ps[:, 0:fw],
                                         w_sb[wname][:, ki, P * i:P * i + P],
                                         rhs_sb[:, ki, tt:tt + fw],
                                         start=(ki == 0), stop=(ki == KI - 1))
                    if has_b[wname]:
                        nc.scalar.activation(out_tiles[i][:, tt:tt + fw], ps[:, 0:fw],
                                             AF.Identity, bias=bv_sb[wname][:, i:i + 1])
                    else:
                        nc.vector.tensor_copy(out_tiles[i][:, tt:tt + fw], ps[:, 0:fw])

        # V in natural layout, ones-augmented per head: [tok, H*(D+1)]
        def vproj(wname, src_sb, rows, row_tiles, out_tiles, ones_rows=None):
            for m in range(row_tiles):
                pr = min(P, rows - m * P)
                ones_r = pr if ones_rows is None else min(ones_rows, pr)
                ps = ps_b.tile([P, 512], f32, tag="psb")
                if pr < P:
                    nc.vector.memset(out_tiles[m][:], 0.0)
                for ki in range(KI):
                    nc.tensor.matmul(ps[0:pr, :],
                                     src_sb[:, ki, m * P:m * P + pr],
                                     w_sb[wname][:, ki, :],
                                     start=(ki == 0), stop=(ki == KI - 1))
                dst = out_tiles[m].rearrange("p (h e) -> p h e", e=DA)
                nc.vector.tensor_copy(dst[0:pr, :, 0:D],
                                      ps[0:pr, :].rearrange("p (h e) -> p h e", e=D))
                if has_b[wname]:
                    nc.vector.tensor_tensor(
                        dst[0:pr, :, 0:D], dst[0:pr, :, 0:D],
                        bv_sb[wname][0:pr, :].rearrange("p (h e) -> p h e", e=D),
                        ALU.add)
                if ones_r < pr:
                    nc.vector.memset(dst[:, :, D:DA], 0.0)
                nc.vector.memset(dst[0:ones_r, :, D:DA], 1.0)

        def projections():
            proj_T("q", xqT_sb, TQ, qT_sb, 512)
            proj_T("k", xT_sb, T, kT_sb, 512)
            # c zero-padded to MP=128 tokens on host -> 128-clean cross
            # shapes; padded K_c/V_c columns are zero, junk logit rows see
            # exp(0)=1 but multiply against zero V_c rows + zero ones-col.
            proj_T("kc", cT_sb, MP, kcT_sb, MP)
            vproj("v", xT_sb, T, T // P, v_sb)
            vproj("vc", cT_sb, MP, 1, vc_sb, ones_rows=M)

        # ---- attention ----
        def attention():
            for i in range(PAIRS):
                for pos in range(2):
                    q0 = pos * QC
                    ngrp = ext[pos] // GROUP
                    yps = ps_y.tile([DA, 2 * QC], f32, tag="y")
                    for hb in range(2):  # heads 2i (rows 0:64) / 2i+1 (64:128)
                        b0 = hb * D
                        yslice = yps[:, hb * QC:(hb + 1) * QC]

                        def qk_group(g):
                            # logits for k-tiles [4g, 4g+4); masking is a 0/1
                            # multiply on the exp output, run on the otherwise
                            # idle GPSIMD engine (PE does pure QK matmuls)
                            lg = ps_a.tile([P, GROUP * QC], f32, tag="lg")
                            for s4 in range(GROUP):
                                s = g * GROUP + s4
                                nc.tensor.matmul(
                                    lg[:, s4 * QC:(s4 + 1) * QC],
                                    kT_sb[i][b0:b0 + D, s * KT:(s + 1) * KT],
                                    qT_sb[i][b0:b0 + D, q0:q0 + QC],
                                    start=True, stop=True)
                            pt = work.tile([P, GROUP * QC], f16, tag="pt")
                            nc.scalar.activation(pt[:], lg[:], AF.Exp)
                            if (pos, g) in bias_idx:
                                nc.vector.tensor_tensor(
                                    pt[:], pt[:],
                                    bias_sb[:, bias_idx[(pos, g)], :], ALU.mult)
                            return pt

                        def av_group(g, pt):
                            for s4 in range(GROUP):
                                s = g * GROUP + s4
                                nc.tensor.matmul(
                                    yslice,
                                    v_sb[s][:, (2 * i + hb) * DA:(2 * i + hb + 1) * DA],
                                    pt[:, s4 * QC:(s4 + 1) * QC],
                                    start=(s == 0), stop=(s == ext[pos] - 1))

                        # lag-2 software pipeline: QK(g+2) is emitted before
                        # AV(g) so the exp(g) latency hides behind PE work
                        pts = {}
                        for g in range(ngrp):
                            pts[g] = qk_group(g)
                            if g >= 2:
                                av_group(g - 2, pts.pop(g - 2))
                        for g in range(max(0, ngrp - 2), ngrp):
                            av_group(g, pts.pop(g))
                    # cross-attention for this (pair, chunk)
                    branches = [(yps, yT_sb)]
                    do_cqk = stage in (31, 32, 3, 4)
                    do_avc = stage in (32, 3, 4)
                    do_cnorm = stage in (3, 4)
                    if do_cqk:
                        # separate PSUM tiles (banks) per head: the two S_c
                        # matmuls run concurrently in different PE row-groups
                        # and must not drain into the same PSUM bank.
                        scp = [ps_b.tile([P, QC], f32, tag="psb", name=f"scp{hb}")
                               for hb in range(2)]
                        pct = work.tile([P, 2 * QC], f16, tag="pct")
                        for hb in range(2):
                            b0 = hb * D
                            nc.tensor.matmul(scp[hb][:, 0:QC],
                                             kcT_sb[i][b0:b0 + D, :],
                                             qT_sb[i][b0:b0 + D, q0:q0 + QC],
                                             start=True, stop=True)
                            nc.scalar.activation(pct[:, hb * QC:(hb + 1) * QC],
                                                 scp[hb][:, 0:QC], AF.Exp,
                                                 bias=pad_sb[:, 0:1])
                        if not do_avc:
                            nc.vector.tensor_copy(ycT_sb[i][:, q0:q0 + QC],
                                                  pct[:, 0:QC])
                    if do_avc:
                        ycps = ps_y.tile([DA, 2 * QC], f32, tag="y")
                        for hb in range(2):
                            nc.tensor.matmul(
                                ycps[:, hb * QC:(hb + 1) * QC],
                                vc_sb[0][:, (2 * i + hb) * DA:(2 * i + hb + 1) * DA],
                                pct[:, hb * QC:(hb + 1) * QC],
                                start=True, stop=True)
                        if not do_cnorm:
                            nc.vector.tensor_copy(
                                ycT_sb[i][0:DA, q0:q0 + QC], ycps[:, 0:QC])
                    if do_cnorm:
                        branches.append((ycps, ycT_sb))
                    # normalize branches into yT/ycT
                    for ps, dst in branches:
                        rec = nrm.tile([1, 2 * QC], f32, tag="rec")
                        nc.vector.reciprocal(rec[:], ps[D:DA, :])
                        bc = nrm.tile([D, 2 * QC], f32, tag="bc")
                        nc.sync.dma_start(
                            bc[:],
                            rec[0:1, :].unsqueeze(1).to_broadcast((1, D, 2 * QC)))
                        for hb in range(2):
                            nc.vector.tensor_tensor(
                                dst[i][hb * D:(hb + 1) * D, q0:q0 + QC],
                                ps[0:D, hb * QC:(hb + 1) * QC],
                                bc[:, hb * QC:(hb + 1) * QC], ALU.mult)

        # ---- gates, combine, output projection ----
        def gates_out():
            for o in range(PAIRS):
                for wname, src, dst, bn in (("g1", yT_sb, g1_sb, "g1"),
                                            ("g2", ycT_sb, g2_sb, "g2")):
                    ps = ps_b.tile([P, 512], f32, tag="psb")
                    for i in range(PAIRS):
                        nc.tensor.matmul(ps[:], w_sb[wname][:, i, P * o:P * o + P],
                                         src[i][:], start=(i == 0),
                                         stop=(i == PAIRS - 1))
                    bias = bv_sb[bn][:, o:o + 1] if has_b[bn] else 0.0
                    nc.scalar.activation(dst[o][:], ps[:], AF.Sigmoid, bias=bias)
                t1 = work.tile([P, TQ], f16, tag="zt")
                nc.vector.tensor_tensor(t1[:], g1_sb[o][:], ycT_sb[o][:], ALU.mult)
                nc.vector.tensor_tensor(z_sb[o][:], g2_sb[o][:], yT_sb[o][:], ALU.mult)
                nc.vector.tensor_tensor(z_sb[o][:], z_sb[o][:], t1[:], ALU.add)
            for m in range(PAIRS):
                ps = ps_b.tile([P, 512], f32, tag="psb")
                for o in range(PAIRS):
                    nc.tensor.matmul(ps[:], z_sb[o][:, P * m:P * m + P],
                                     w_sb["p"][:, o, :], start=(o == 0),
                                     stop=(o == PAIRS - 1))
                osb = work.tile([P, C], f32, tag="osb")
                if has_b["p"]:
                    nc.vector.tensor_tensor(osb[:], ps[:], bv_sb["p"][:], ALU.add)
                else:
                    nc.vector.tensor_copy(osb[:], ps[:])
                nc.sync.dma_start(out_d[P * m:P * m + P, :], osb[:])

        for rep in range(max(1, repeat)):
            projections()
            if stage == 1:
                dump([qT_sb[0][:, 0:C], kT_sb[0][:, 0:C],
                      v_sb[0][:, 0:C], vc_sb[0][:, 0:C]])
                return
            if stage == 15:
                dump([t[:] for t in kcT_sb])
                return
            attention()
            if stage in (2, 31, 32):
                dump([t[:, 0:C] for t in yT_sb])
                return
            if stage == 3:
                dump([t[:, 0:C] for t in ycT_sb])
                return
            gates_out()

    with tile.TileContext(nc) as tc, ExitStack() as ctx:
        emit(tc, ctx)
    nc.compile()
    _cache[key] = nc
    return nc


def prepare(inputs, stage=4, repeat=1):
    """Host-side prep: analyze mask, build program + per-core input maps."""
    x = np.asarray(inputs["x"], np.float32)
    c = np.asarray(inputs["c"], np.float32)
    attn_mask = np.asarray(inputs["attn_mask"])
    padding_mask = np.asarray(inputs["padding_mask"])
    W = {n: np.asarray(inputs["W" + n], np.float32)
         for n in ["q", "k", "v", "kc", "vc", "g1", "g2", "p"]}
    bvec = {n: np.asarray(inputs["b" + n], np.float32)
            for n in ["q", "k", "v", "kc", "vc", "g1", "g2", "p"]}

    scale = 1.0 / np.sqrt(D)
    W = dict(W)
    W["q"] = W["q"] * scale          # fold attention scale into Wq
    bq = bvec["q"] * scale

    mask2 = np.asarray(attn_mask).reshape(T, T)  # [q, k]
    ext_chunk = []
    for qc in range(NCHUNK):
        vis = mask2[qc * QC:(qc + 1) * QC, :].any(axis=0)
        last = int(np.nonzero(vis)[0].max()) if vis.any() else 0
        ext_chunk.append(last // KT + 1)
    ext = []
    for pos in range(2):
        e = max(ext_chunk[_chunks_of_core(j)[pos]] for j in range(4))
        ext.append(-(-e // GROUP) * GROUP)
    def _slot_needs(pos, s):
        for j in range(4):
            qc = _chunks_of_core(j)[pos]
            if s >= ext_chunk[qc]:
                return True
            blk = mask2[qc * QC:(qc + 1) * QC, s * KT:(s + 1) * KT]
            if not blk.all():
                return True
        return False

    # mask "units" cover one GROUP of k-slots (0/1 multiply on exp output)
    bias_slots = []
    for pos in range(2):
        for g in range(ext[pos] // GROUP):
            if any(_slot_needs(pos, g * GROUP + s4) for s4 in range(GROUP)):
                bias_slots.append((pos, g))

    has_b = {n: bool(np.any(bvec[n] != 0)) for n in bvec}
    nc = build_program(ext, bias_slots, has_b, stage=stage, repeat=repeat)

    w16 = {n: W[n].astype(np.float16) for n in W}
    ident = np.eye(P, dtype=np.float16)
    in_maps = []
    for core in range(8):
        b, j = divmod(core, 4)
        ca, cb = _chunks_of_core(j)
        xT = np.ascontiguousarray(x[b].T).astype(np.float16)        # [C, T]
        cols = np.r_[ca * QC:(ca + 1) * QC, cb * QC:(cb + 1) * QC]
        xqT = np.ascontiguousarray(xT[:, cols])
        cT = np.zeros((C, MP), np.float16)
        cT[:, :M] = c[b].T
        pad = np.zeros((P, 1), np.float32)
        pad[:M, 0] = np.where(padding_mask[b] != 0, 0.0, NEG)
        im = {"xT": xT, "xqT": xqT, "cT": cT, "ident": ident, "padb": pad}
        for n in w16:
            im["w" + n] = w16[n]
        if bias_slots:
            bm = np.empty((len(bias_slots), P, GROUP * QC), np.float16)
            for n, (pos, g) in enumerate(bias_slots):
                qc = (ca, cb)[pos]
                for e in range(GROUP):
                    s = g * GROUP + e
                    blk = mask2[qc * QC:(qc + 1) * QC, s * KT:(s + 1) * KT]
                    bm[n, :, e * QC:(e + 1) * QC] = np.where(
                        blk.T, 1.0, 0.0).astype(np.float16)
            im["biasm"] = bm
        for n in ["q", "k", "kc", "g1", "g2"]:
            if has_b[n]:
                v = (bq if n == "q" else bvec[n])
                im["b" + n] = np.ascontiguousarray(
                    v.reshape(KI, P).T).astype(np.float32)
        for n in ["v", "vc", "p"]:
            if has_b[n]:
                im["b" + n] = bvec[n].reshape(1, C).astype(np.float16)
        in_maps.append(im)
    return nc, in_maps


def kernel(**inputs):
    nc, in_maps = prepare(inputs)
    res = bass_utils.run_bass_kernel_spmd(nc, in_maps, core_ids=list(range(8)))
    out = np.empty((B, T, C), np.float32)
    for core in range(8):
        b, j = divmod(core, 4)
        ca, cb = _chunks_of_core(j)
        o = res.results[core]["out"]
        out[b, ca * QC:(ca + 1) * QC] = o[:QC]
        out[b, cb * QC:(cb + 1) * QC] = o[QC:]
    return out



# revision 6
# speedup vs baseline: 1.1228x; 1.1228x over previous
"""ConditionGateAttention Trainium2 kernel.

Gated dual-attention block: causal self-attention + cross-attention to a
77-token condition, sigmoid cross-gating, output projection.

  B=2, T=2048, M=77, C=512, H=8 heads, D=64.

Sharding (8 cores): batch x strided-sequence. Core = (b=core//4, j=core%4);
core owns query rows j::4 of its batch, viewed as 4 local blocks of 128
contiguous local rows (= global rows 4*l+j). Under the causal mask every
block bk needs exactly 4*(bk+1) k-tiles on EVERY core, so the program is
SPMD-uniform with zero cross-core padding waste (the baseline chunk-pair
scheme padded 24 tiles vs the ideal 18 per 512 queries).

Attention output is computed token-major ([q, d] PSUM): the AV matmul uses
the softmax tile as stationary and V as moving (free dim 65 instead of
256), and softmax denominators (ones-column of V) land per-partition so
normalization is a tiny reciprocal + per-head tensor_scalar multiply.
The [C, tok] layout needed by the gate matmuls is rebuilt with PE
transposes (128x128 identity matmuls), cheap on the tensor engine.
PSUM-draining copies run on the otherwise idle GPSIMD engine.

Masking: only diagonal k-groups need mask tiles (0/1 multiply on the exp
output); host derives extents and mask data from attn_mask generically.
padding_mask becomes a per-partition ACT bias on the cross-attention exp.
Matmul inputs fp16 (full PE rate), fp32 PSUM accumulate.
"""
import numpy as np
import ml_dtypes
from contextlib import ExitStack

import concourse.bass as bass
import concourse.tile as tile
from concourse import bacc, mybir
from concourse import bass_utils

B, T, M, C, H = 2, 2048, 77, 512, 8
D = C // H            # 64
P = 128
KI = C // P           # 4 contraction chunks
PAIRS = H // 2        # 4 head pairs (pair i = heads 2i, 2i+1 = C rows 128i..128i+128)
TQ = T // 4           # local queries per core (512)
NB = TQ // P          # 4 query blocks of 128 local rows
KT = 128              # k-tile size (partition dim of logits)
GROUP = 4             # k-tiles per logits psum group
NEG = -30000.0        # mask bias (exp(-30000+s) == 0)
MP = 128              # condition length M=77 zero-padded to 128 on host
DA = D + 1            # V augmented with a ones-column (denominator col)

f16 = mybir.dt.float16
f32 = mybir.dt.float32
AF = mybir.ActivationFunctionType
ALU = mybir.AluOpType

_cache = {}


def build_program(ext, bias_slots, has_b, stage=4):
    """ext: per-block k-extent in KT tiles (uniform across cores), rounded
    up to GROUP. bias_slots: list of (bk, g) needing a mask tile (uniform;
    data per-core). has_b: dict of which projection biases are nonzero.
    stage: 0=io, 1=projections, 15=kc, 2=+self-attn, 3=+cross, 4=full."""
    key = (tuple(ext), tuple(bias_slots), tuple(sorted(has_b.items())), stage)
    if key in _cache:
        return _cache[key]

    nb = len(bias_slots)
    bias_idx = {ps: n for n, ps in enumerate(bias_slots)}

    nc = bacc.Bacc("TRN2", num_devices=8, debug=False)

    xT_d = nc.dram_tensor("xT", [C, T], f16, kind="ExternalInput").ap()
    xqT_d = nc.dram_tensor("xqT", [C, TQ], f16, kind="ExternalInput").ap()
    cT_d = nc.dram_tensor("cT", [C, MP], f16, kind="ExternalInput").ap()
    w_d = {n: nc.dram_tensor(f"w{n}", [C, C], f16, kind="ExternalInput").ap()
           for n in ["q", "k", "v", "kc", "vc", "g1", "g2", "p"]}
    ident_d = nc.dram_tensor("ident", [P, P], f16, kind="ExternalInput").ap()
    pad_d = nc.dram_tensor("padb", [P, 1], f32, kind="ExternalInput").ap()
    if nb:
        bias_d = nc.dram_tensor("biasm", [nb, P, GROUP * KT], f16,
                                kind="ExternalInput").ap()
    bv_d = {}
    for n in ["q", "k", "kc", "g1", "g2"]:
        if has_b[n]:
            bv_d[n] = nc.dram_tensor(f"b{n}", [P, KI], f32, kind="ExternalInput").ap()
    for n in ["v", "vc", "p"]:
        if has_b[n]:
            bv_d[n] = nc.dram_tensor(f"b{n}", [1, C], f16, kind="ExternalInput").ap()
    out_d = nc.dram_tensor("out", [TQ, C], f32, kind="ExternalOutput").ap()

    def emit(tc, ctx):
        consts = ctx.enter_context(tc.tile_pool(name="consts", bufs=1))
        acts = ctx.enter_context(tc.tile_pool(name="acts", bufs=1))
        work = ctx.enter_context(tc.tile_pool(name="work", bufs=4))
        nrm = ctx.enter_context(tc.tile_pool(name="nrm", bufs=4))
        # PSUM budget (8 banks): ps [P,512]f32 x3 + y [P,260]f32 x3 + tp x2
        ps_p = ctx.enter_context(tc.tile_pool(name="ps_p", bufs=3, space="PSUM"))
        ps_y = ctx.enter_context(tc.tile_pool(name="ps_y", bufs=3, space="PSUM"))
        ps_t = ctx.enter_context(tc.tile_pool(name="ps_t", bufs=2, space="PSUM"))

        # ---- load constants/inputs ----
        def chunked(ap):  # [C, n] dram -> [128, 4, n] view
            return ap.rearrange("(o p) n -> p o n", p=P)

        # DMA order matters: q-projection inputs land first (split per
        # contraction chunk so PE starts after 1/4 of the bytes), then the
        # rest streams in roughly in consumption order.
        w_sb = {n: consts.tile([P, KI, C], f16, name=f"w{n}") for n in w_d}
        xqT_sb = consts.tile([P, KI, TQ], f16, name="xqT")
        xT_sb = consts.tile([P, KI, T], f16, name="xT")
        for ki in range(KI):
            nc.sync.dma_start(xqT_sb[:, ki], chunked(xqT_d)[:, ki])
            nc.sync.dma_start(w_sb["q"][:, ki], chunked(w_d["q"])[:, ki])
        ident = consts.tile([P, P], f16, name="ident")
        nc.sync.dma_start(ident[:], ident_d)
        cT_sb = consts.tile([P, KI, MP], f16, name="cT")
        nc.sync.dma_start(cT_sb[:], chunked(cT_d))
        nc.sync.dma_start(w_sb["kc"][:], chunked(w_d["kc"]))
        pad_sb = consts.tile([P, 1], f32, name="padb")
        nc.sync.dma_start(pad_sb[:], pad_d)
        nc.sync.dma_start(w_sb["k"][:], chunked(w_d["k"]))
        for ki in range(KI):
            nc.sync.dma_start(xT_sb[:, ki], chunked(xT_d)[:, ki])
        for n in ["vc", "v", "g1", "g2", "p"]:
            nc.sync.dma_start(w_sb[n][:], chunked(w_d[n]))
        if nb:
            bias_sb = consts.tile([P, nb, GROUP * KT], f16, name="biasm")
            nc.sync.dma_start(bias_sb[:], bias_d.rearrange("n p q -> p n q"))
        bv_sb = {}
        for n, d in bv_d.items():
            if n in ("v", "vc", "p"):
                bv_sb[n] = consts.tile([P, C], f16, name=f"b{n}")
                nc.sync.dma_start(bv_sb[n][:],
                                  d[0:1, :].unsqueeze(1).to_broadcast((1, P, C)))
            else:
                bv_sb[n] = consts.tile([P, KI], f32, name=f"b{n}")
                nc.sync.dma_start(bv_sb[n][:], d)

        def dump(srcs):
            for m, src in enumerate(srcs):
                osb = work.tile([P, C], f32, tag="osb")
                w = src.shape[-1]
                if w < C:
                    nc.vector.memset(osb[:], 0.0)
                nc.vector.tensor_copy(osb[:, 0:w], src)
                nc.sync.dma_start(out_d[P * m:P * m + P, :], osb[:])

        if stage == 0:
            for m in range(NB):
                osb = work.tile([P, C], f32, tag="osb")
                nc.vector.memset(osb[:], 0.0)
                nc.sync.dma_start(out_d[P * m:P * m + P, :], osb[:])
            return

        # ---- persistent activation tiles ----
        qT_sb = [acts.tile([P, TQ], f16, name=f"qT{i}") for i in range(PAIRS)]
        kT_sb = [acts.tile([P, T], f16, name=f"kT{i}") for i in range(PAIRS)]
        kcT_sb = [acts.tile([P, MP], f16, name=f"kcT{i}") for i in range(PAIRS)]
        v_sb = [acts.tile([P, H * DA], f16, name=f"v{m}") for m in range(T // P)]
        vc_sb = [acts.tile([P, H * DA], f16, name="vc")]
        pct_sb = [acts.tile([P, 2 * TQ], f16, name=f"pct{i}") for i in range(PAIRS)]
        yT_sb = [acts.tile([P, TQ], f16, name=f"yT{i}") for i in range(PAIRS)]
        ycT_sb = [acts.tile([P, TQ], f16, name=f"ycT{i}") for i in range(PAIRS)]
        g1_sb = [acts.tile([P, TQ], f16, name=f"g1_{o}") for o in range(PAIRS)]
        g2_sb = [acts.tile([P, TQ], f16, name=f"g2_{o}") for o in range(PAIRS)]
        z_sb = [acts.tile([P, TQ], f16, name=f"z{o}") for o in range(PAIRS)]

        # ---- projections ----
        def proj_T(wname, rhs_sb, n_free, out_tiles, free_tile):
            # out[Cout, n] = W.T @ actT ; out_tiles[i] [128, n_free] f16
            for i in range(PAIRS):
                for tt in range(0, n_free, free_tile):
                    fw = min(free_tile, n_free - tt)
                    ps = ps_p.tile([P, 512], f32, tag="ps")
                    for ki in range(KI):
                        nc.tensor.matmul(ps[:, 0:fw],
                                         w_sb[wname][:, ki, P * i:P * i + P],
                                         rhs_sb[:, ki, tt:tt + fw],
                                         start=(ki == 0), stop=(ki == KI - 1))
                    if has_b[wname]:
                        nc.scalar.activation(out_tiles[i][:, tt:tt + fw], ps[:, 0:fw],
                                             AF.Identity, bias=bv_sb[wname][:, i:i + 1])
                    else:
                        nc.vector.tensor_copy(out_tiles[i][:, tt:tt + fw], ps[:, 0:fw])

        # V in natural layout, ones-augmented per head: [tok, H*(D+1)]
        def vproj(wname, src_sb, rows, row_tiles, out_tiles, ones_rows=None):
            for m in range(row_tiles):
                pr = min(P, rows - m * P)
                ones_r = pr if ones_rows is None else min(ones_rows, pr)
                ps = ps_p.tile([P, 512], f32, tag="ps")
                if pr < P:
                    nc.gpsimd.memset(out_tiles[m][:], 0.0)
                for ki in range(KI):
                    nc.tensor.matmul(ps[0:pr, :],
                                     src_sb[:, ki, m * P:m * P + pr],
                                     w_sb[wname][:, ki, :],
                                     start=(ki == 0), stop=(ki == KI - 1))
                dst = out_tiles[m].rearrange("p (h e) -> p h e", e=DA)
                nc.vector.tensor_copy(dst[0:pr, :, 0:D],
                                      ps[0:pr, :].rearrange("p (h e) -> p h e", e=D))
                if has_b[wname]:
                    nc.gpsimd.tensor_tensor(
                        dst[0:pr, :, 0:D], dst[0:pr, :, 0:D],
                        bv_sb[wname][0:pr, :].rearrange("p (h e) -> p h e", e=D),
                        ALU.add)
                if ones_r < pr:
                    nc.gpsimd.memset(dst[:, :, D:DA], 0.0)
                nc.gpsimd.memset(dst[0:ones_r, :, D:DA], 1.0)

        def projections():
            proj_T("q", xqT_sb, TQ, qT_sb, TQ)
            proj_T("kc", cT_sb, MP, kcT_sb, MP)
            proj_T("k", xT_sb, T, kT_sb, 512)
            vproj("vc", cT_sb, MP, 1, vc_sb, ones_rows=M)
            vproj("v", xT_sb, T, T // P, v_sb)

        # cross-attention logits+exp for all local queries, per (pair, head);
        # overlaps the k/v projections. c zero-padded to MP=128 tokens on
        # host: padded K_c/V_c columns are zero, junk logit rows see
        # exp(0)=1 but multiply against zero V_c rows + zero ones-col.
        def cross_logits():
            for i in range(PAIRS):
                for hb in range(2):
                    b0 = hb * D
                    ps = ps_p.tile([P, 512], f32, tag="ps")
                    nc.tensor.matmul(ps[:, 0:TQ],
                                     kcT_sb[i][b0:b0 + D, :],
                                     qT_sb[i][b0:b0 + D, :],
                                     start=True, stop=True)
                    nc.scalar.activation(pct_sb[i][:, hb * TQ:(hb + 1) * TQ],
                                         ps[:, 0:TQ], AF.Exp,
                                         bias=pad_sb[:, 0:1])

        # ---- attention for one query block ----
        def normalize(pslo, pshi, dst_sb):
            # ps [128q, 4*DA] f32 x2: cols h*DA+64 are denominators.
            lo3 = pslo.rearrange("p (h e) -> p h e", e=DA)
            hi3 = pshi.rearrange("p (h e) -> p h e", e=DA)
            rec = nrm.tile([P, H], f32, tag="rec")
            nc.vector.reciprocal(rec[:, 0:4], lo3[:, :, D:DA])
            nc.vector.reciprocal(rec[:, 4:8], hi3[:, :, D:DA])
            for idx, src3 in ((0, lo3), (1, hi3)):
                nc.vector.tensor_tensor(
                    dst_sb[:, idx * 4 * D:(idx + 1) * 4 * D].rearrange(
                        "p (h e) -> p h e", e=D),
                    src3[:, :, 0:D],
                    rec[:, idx * 4:(idx + 1) * 4].unsqueeze(2).to_broadcast(
                        (P, 4, D)),
                    ALU.mult)

        def transposes(src_sb, dst_tiles, bk):
            for c4 in range(KI):
                tps = ps_t.tile([P, P], f16, tag="tp")
                nc.tensor.transpose(tps[:], src_sb[:, c4 * P:(c4 + 1) * P], ident[:])
                nc.vector.tensor_copy(dst_tiles[c4][:, bk * P:(bk + 1) * P], tps[:])

        def attention_block(bk, do_cross):
            q0 = bk * P
            ngrp = ext[bk] // GROUP
            ylo = ps_y.tile([P, 4 * DA], f32, tag="y")
            yhi = ps_y.tile([P, 4 * DA], f32, tag="y")

            def yslice(h):
                t3 = (ylo if h < 4 else yhi).rearrange("p (h e) -> p h e", e=DA)
                return t3[:, h % 4, :]

            def qk_group(i, hb, g):
                # logits for k-tiles [4g, 4g+4) of head 2i+hb
                b0 = hb * D
                lg = ps_p.tile([P, 512], f32, tag="ps")
                for s4 in range(GROUP):
                    s = g * GROUP + s4
                    nc.tensor.matmul(
                        lg[:, s4 * KT:(s4 + 1) * KT],
                        kT_sb[i][b0:b0 + D, s * KT:(s + 1) * KT],
                        qT_sb[i][b0:b0 + D, q0:q0 + P],
                        start=True, stop=True)
                pt = work.tile([P, 512], f16, tag="pt")
                nc.scalar.activation(pt[:], lg[:], AF.Exp)
                if (bk, g) in bias_idx:
                    # big blocks have pipeline slack: run the 0/1 mask
                    # multiply on the idle GPSIMD engine there; small
                    # blocks sit on the critical QK->AV chain, keep on DVE.
                    eng = nc.gpsimd if bk >= 2 else nc.vector
                    eng.tensor_tensor(
                        pt[:], pt[:], bias_sb[:, bias_idx[(bk, g)], :], ALU.mult)
                return pt

            def av_group(i, hb, g, pt):
                h = 2 * i + hb
                for s4 in range(GROUP):
                    s = g * GROUP + s4
                    nc.tensor.matmul(
                        yslice(h),
                        pt[:, s4 * KT:(s4 + 1) * KT],
                        v_sb[s][:, h * DA:(h + 1) * DA],
                        start=(s == 0), stop=(s == ext[bk] - 1))

            # lag-2 software pipeline over the flattened (pair, head, group)
            # stream: QK(u) is emitted before AV(u-2) so exp latency hides
            # behind PE work.
            units = [(i, hb, g) for i in range(PAIRS) for hb in range(2)
                     for g in range(ngrp)]
            pts = {}
            for u, (i, hb, g) in enumerate(units):
                pts[u] = (i, hb, g, qk_group(i, hb, g))
                if u >= 2:
                    iu, hu, gu, pt = pts.pop(u - 2)
                    av_group(iu, hu, gu, pt)
            for u in sorted(pts):
                iu, hu, gu, pt = pts.pop(u)
                av_group(iu, hu, gu, pt)

            if do_cross:
                yclo = ps_y.tile([P, 4 * DA], f32, tag="y")
                ychi = ps_y.tile([P, 4 * DA], f32, tag="y")

                def ycslice(h):
                    t3 = (yclo if h < 4 else ychi).rearrange(
                        "p (h e) -> p h e", e=DA)
                    return t3[:, h % 4, :]

                for i in range(PAIRS):
                    for hb in range(2):
                        h = 2 * i + hb
                        nc.tensor.matmul(
                            ycslice(h),
                            pct_sb[i][:, hb * TQ + q0: hb * TQ + q0 + P],
                            vc_sb[0][:, h * DA:(h + 1) * DA],
                            start=True, stop=True)

            ysb = work.tile([P, C], f16, tag="ysb")
            normalize(ylo, yhi, ysb)
            transposes(ysb, yT_sb, bk)
            if do_cross:
                ycsb = work.tile([P, C], f16, tag="ysb")
                normalize(yclo, ychi, ycsb)
                transposes(ycsb, ycT_sb, bk)

        # ---- gates, combine, output projection ----
        def gates_out():
            for o in range(PAIRS):
                for wname, src, dst, bn in (("g1", yT_sb, g1_sb, "g1"),
                                            ("g2", ycT_sb, g2_sb, "g2")):
                    ps = ps_p.tile([P, 512], f32, tag="ps")
                    for i in range(PAIRS):
                        nc.tensor.matmul(ps[:],
                                         w_sb[wname][:, i, P * o:P * o + P],
                                         src[i][:], start=(i == 0),
                                         stop=(i == PAIRS - 1))
                    bias = bv_sb[bn][:, o:o + 1] if has_b[bn] else 0.0
                    nc.scalar.activation(dst[o][:], ps[:], AF.Sigmoid, bias=bias)
                t1 = work.tile([P, TQ], f16, tag="zt")
                nc.vector.tensor_tensor(t1[:], g1_sb[o][:], ycT_sb[o][:], ALU.mult)
                nc.vector.tensor_tensor(z_sb[o][:], g2_sb[o][:], yT_sb[o][:], ALU.mult)
                nc.vector.tensor_tensor(z_sb[o][:], z_sb[o][:], t1[:], ALU.add)
            for m in range(NB):
                ps = ps_p.tile([P, 512], f32, tag="ps")
                for o in range(PAIRS):
                    nc.tensor.matmul(ps[:], z_sb[o][:, P * m:P * m + P],
                                     w_sb["p"][:, o, :], start=(o == 0),
                                     stop=(o == PAIRS - 1))
                osb = work.tile([P, C], f32, tag="osb")
                if has_b["p"]:
                    nc.vector.tensor_tensor(osb[:], ps[:], bv_sb["p"][:], ALU.add)
                else:
                    nc.vector.tensor_copy(osb[:], ps[:])
                nc.sync.dma_start(out_d[P * m:P * m + P, :], osb[:])

        projections()
        if stage == 1:
            dump([qT_sb[0][:, 0:C], kT_sb[0][:, 0:C],
                  v_sb[0][:, 0:C], vc_sb[0][:, 0:C]])
            return
        if stage == 15:
            dump([t[:] for t in kcT_sb])
            return
        do_cross = stage in (3, 4)
        if do_cross:
            cross_logits()
        # big blocks first: sustained PE work early (p-state ramp), the
        # smallest block's short tail feeds straight into gates_out.
        for bk in range(NB - 1, -1, -1):
            attention_block(bk, do_cross)
        if stage == 2:
            dump([t[:] for t in yT_sb])
            return
        if stage == 3:
            dump([t[:] for t in ycT_sb])
            return
        gates_out()

    with tile.TileContext(nc) as tc, ExitStack() as ctx:
        emit(tc, ctx)
    nc.compile()
    _cache[key] = nc
    return nc


def prepare(inputs, stage=4):
    """Host-side prep: analyze mask, build program + per-core input maps."""
    x = np.asarray(inputs["x"], np.float32)
    c = np.asarray(inputs["c"], np.float32)
    attn_mask = np.asarray(inputs["attn_mask"])
    padding_mask = np.asarray(inputs["padding_mask"])
    W = {n: np.asarray(inputs["W" + n], np.float32)
         for n in ["q", "k", "v", "kc", "vc", "g1", "g2", "p"]}
    bvec = {n: np.asarray(inputs["b" + n], np.float32)
            for n in ["q", "k", "v", "kc", "vc", "g1", "g2", "p"]}

    scale = 1.0 / np.sqrt(D)
    W = dict(W)
    W["q"] = W["q"] * scale          # fold attention scale into Wq
    bq = bvec["q"] * scale

    mask2 = np.asarray(attn_mask).reshape(T, T)  # [q, k]
    # local row l of core j = global row 4*l+j; block bk = local rows
    # [128*bk, 128*bk+128). Extents are maxed over cores (program-uniform).
    rows_of = {j: np.arange(j, T, 4) for j in range(4)}
    ext = []
    last_vis = {}
    for bk in range(NB):
        e = 0
        for j in range(4):
            rr = rows_of[j][bk * P:(bk + 1) * P]
            vis = mask2[rr, :].any(axis=0)
            last = int(np.nonzero(vis)[0].max()) if vis.any() else 0
            last_vis[(bk, j)] = last
            e = max(e, last // KT + 1)
        ext.append(-(-e // GROUP) * GROUP)

    def _slot_needs(bk, s):
        for j in range(4):
            if s > last_vis[(bk, j)] // KT:
                return True
            rr = rows_of[j][bk * P:(bk + 1) * P]
            if not mask2[np.ix_(rr, np.arange(s * KT, (s + 1) * KT))].all():
                return True
        return False

    bias_slots = []
    for bk in range(NB):
        for g in range(ext[bk] // GROUP):
            if any(_slot_needs(bk, g * GROUP + s4) for s4 in range(GROUP)):
                bias_slots.append((bk, g))

    has_b = {n: bool(np.any(bvec[n] != 0)) for n in bvec}
    nc = build_program(ext, bias_slots, has_b, stage=stage)

    w16 = {n: W[n].astype(np.float16) for n in W}
    ident = np.eye(P, dtype=np.float16)
    in_maps = []
    for core in range(8):
        b, j = divmod(core, 4)
        xT = np.ascontiguousarray(x[b].T).astype(np.float16)        # [C, T]
        xqT = np.ascontiguousarray(xT[:, j::4])                     # [C, TQ]
        cT = np.zeros((C, MP), np.float16)
        cT[:, :M] = c[b].T
        pad = np.zeros((P, 1), np.float32)
        pad[:M, 0] = np.where(padding_mask[b] != 0, 0.0, NEG)
        im = {"xT": xT, "xqT": xqT, "cT": cT, "ident": ident, "padb": pad}
        for n in w16:
            im["w" + n] = w16[n]
        if bias_slots:
            bm = np.empty((len(bias_slots), P, GROUP * KT), np.float16)
            for n, (bk, g) in enumerate(bias_slots):
                rr = rows_of[j][bk * P:(bk + 1) * P]
                for e in range(GROUP):
                    s = g * GROUP + e
                    blk = mask2[np.ix_(rr, np.arange(s * KT, (s + 1) * KT))]
                    bm[n, :, e * KT:(e + 1) * KT] = np.where(
                        blk.T, 1.0, 0.0).astype(np.float16)
            im["biasm"] = bm
        for n in ["q", "k", "kc", "g1", "g2"]:
            if has_b[n]:
                v = (bq if n == "q" else bvec[n])
                im["b" + n] = np.ascontiguousarray(
                    v.reshape(KI, P).T).astype(np.float32)
        for n in ["v", "vc", "p"]:
            if has_b[n]:
                im["b" + n] = bvec[n].reshape(1, C).astype(np.float16)
        in_maps.append(im)
    return nc, in_maps


def kernel(**inputs):
    nc, in_maps = prepare(inputs)
    res = bass_utils.run_bass_kernel_spmd(nc, in_maps, core_ids=list(range(8)))
    out = np.empty((B, T, C), np.float32)
    for core in range(8):
        b, j = divmod(core, 4)
        out[b, j::4] = res.results[core]["out"]
    return out
